# revision 39
# baseline (speedup 1.0000x reference)
"""TRN2 Bass kernel for nn_Attention_59270548685139.

Custom two-stage-normalized attention, B=8, N=1024, D=1024, H=8, DH=64.
Sharding: data-parallel over batch -- one batch element per NeuronCore (8 cores).

Math per batch element (matching the reference):
  q = x @ Wq, k = x @ Wk, v = x @ Wv          (split into 8 heads of 64)
  sim[i,j]  = (q_i . k_j) * DH**-0.5
  attn      = softmax over the QUERY dim i    -> E[i,j]/C[j], C[j] = sum_i E[i,j]
  attn      = attn / (sum_j attn + eps)       -> per-i scale 1/(R[i]+eps)
  out       = attn @ v ; y = out @ Wo + bo

Schedule (v4): the whole pair loop is paced by the ACT exp stream.
- Scores ring 3-deep ("big" psum); chain (attn@V) i-half-0 links are
  emitted incrementally one jb behind the exps into a held "u" bank per
  head; i-half-1 runs as an 8-matmul burst at the pair boundary into the
  same bank after the half-0 drain, so chains cost no extra PSUM.
- C (softmax denominator): head A's C comes from a DVE reduce over the
  bf16 E tile; head B keeps the ACT accumulator -- splits the per-pair
  ACT load and removes half the READ_ACCUMULATOR stalls.
- finish(h) (1/R normalization of the chain output) runs at the pair's
  own boundary in the freed "u" banks, so the tail only waits on the
  last pair.
- Out-projection: blocks pre-accumulate Wo chunks mb0-2 into psum during
  late pair 3 / the finish window, and drain (+bo) into SBUF partials
  aliased onto the dead qt/kt tiles; after finish(7) only the 16-matmul
  mb3 sweep + per-block adds remain.
- PE fillers (v/qk projections) are emitted before each jb's scores so
  the in-order PE queue never idles behind a slot wait; HAM stays warm.
"""

import os

import numpy as np

import concourse.bass as bass
import concourse.tile as tile
from concourse import bacc, mybir
from concourse.bass_utils import run_bass_kernel_spmd
from concourse.masks import make_identity

FP32 = mybir.dt.float32
FP32R = mybir.dt.float32r
BF16 = mybir.dt.bfloat16

B, N, D = 8, 1024, 1024
H, DH = 8, 64
INNER = H * DH  # 512
SCALE = DH ** -0.5
EPS = 1e-7  # negligible vs R in [0.85, 1.15]; folded out
P = 128
NCORES = 8

_NC_CACHE = None


def _build_nc():
    nc = bacc.Bacc("TRN2", target_bir_lowering=False, debug=False)

    x_d = nc.dram_tensor("x", [N, D], FP32, kind="ExternalInput")
    wq_d = nc.dram_tensor("Wq", [D, INNER], FP32, kind="ExternalInput")
    wk_d = nc.dram_tensor("Wk", [D, INNER], FP32, kind="ExternalInput")
    wv_d = nc.dram_tensor("Wv", [D, INNER], FP32, kind="ExternalInput")
    wo_d = nc.dram_tensor("Wo", [INNER, D], FP32, kind="ExternalInput")
    bo_d = nc.dram_tensor("bo", [D], FP32, kind="ExternalInput")
    y_d = nc.dram_tensor("y", [N, D], FP32, kind="ExternalOutput")

    DC = D // P       # 8 contraction chunks over D
    IC = INNER // P   # 4 chunks over INNER
    NB = N // P       # 8 seq blocks of 128

    with tile.TileContext(nc) as tc:
        const_pool = tc.alloc_tile_pool(name="const", bufs=1)
        xt_pool = tc.alloc_tile_pool(name="xt", bufs=1)
        qt_pool = tc.alloc_tile_pool(name="qt", bufs=1)
        kt_pool = tc.alloc_tile_pool(name="kt", bufs=1)
        v_pool = tc.alloc_tile_pool(name="v", bufs=1)
        ot_pool = tc.alloc_tile_pool(name="ot", bufs=1)
        wv_pool = tc.alloc_tile_pool(name="wv", bufs=1)
        w4_pool = tc.alloc_tile_pool(name="w4", bufs=4)
        xn_pool = tc.alloc_tile_pool(name="xn", bufs=5)
        et_pool = tc.alloc_tile_pool(name="et", bufs=1)
        v2_pool = tc.alloc_tile_pool(name="v2", bufs=1)
        c_pool = tc.alloc_tile_pool(name="cp", bufs=1)
        us_pool = tc.alloc_tile_pool(name="us", bufs=2)
        sm_pool = tc.alloc_tile_pool(name="sm", bufs=2)
        y_pool = tc.alloc_tile_pool(name="yp", bufs=2)
        ps_pool = tc.alloc_tile_pool(name="ps", bufs=2, space="PSUM")

        # ---------------- constants ----------------
        ident = const_pool.tile([P, P], FP32, tag="ident")
        make_identity(nc, ident[:])
        # fp32r copy of the identity for the tail's partial re-add matmuls
        # (fp32r consumers require fp32r-rounded producers).
        identr = const_pool.tile([P, P], FP32R, tag="identr")
        nc.vector.tensor_copy(identr[:], ident[:])
        # early HAM warm-up: a burst of dummy matmuls on the identity itself,
        # issued before any x lands, flips the PE clock gate to 2.4 GHz so
        # the x transposes run at full rate from the first block.
        p_ew = ps_pool.tile([P, 512], FP32, tag="u", name="earlywarm")
        for w in range(24):
            nc.tensor.matmul(
                p_ew[:, (w % 4) * P:(w % 4 + 1) * P], ident[:], ident[:],
                start=True, stop=True,
            )
        # bo row + broadcast (added during the yp partial drains).
        bo_row = const_pool.tile([1, D], FP32, tag="bo_row")
        nc.scalar.dma_start(out=bo_row[:], in_=bo_d.ap()[None, :])
        bo_bc = const_pool.tile([P, D], FP32, tag="bo_bc")
        nc.gpsimd.partition_broadcast(bo_bc[:], bo_row[:])
        # preload the Exp table while ACT is idle (table switch is ~1.3us)
        warm_act = const_pool.tile([1, 8], FP32, tag="warm_act")
        nc.scalar.activation(
            warm_act[:], ident[0:1, 0:8], mybir.ActivationFunctionType.Exp
        )

        # ---------------- persistent intermediates ----------------
        xt = [xt_pool.tile([P, N], FP32R, tag=f"xt{c}", name=f"xt{c}") for c in range(DC)]
        qt = [qt_pool.tile([P, N], FP32R, tag=f"qt{m}", name=f"qt{m}") for m in range(IC)]
        kt = [kt_pool.tile([P, N], FP32R, tag=f"kt{m}", name=f"kt{m}") for m in range(IC)]
        vts = [v_pool.tile([P, INNER], FP32, tag=f"v{j}", name=f"v{j}") for j in range(NB)]
        ot = [ot_pool.tile([P, N], FP32R, tag=f"ot{m}", name=f"ot{m}") for m in range(IC)]

        def load_qk_quarter(key, wd, mb, eng=None):
            w4 = w4_pool.tile([P, DC, P], FP32R, tag="w4", name=f"w4{key}{mb}")
            (eng or nc.scalar).dma_start(
                out=w4[:],
                in_=wd.ap()[:, mb * P:(mb + 1) * P]
                .rearrange("(c p) n -> p c n", p=P).bitcast(FP32R),
            )
            return w4

        # ---------------- preamble ----------------
        w4q = {}

        def emit_x_block(ib, n_warm, xb):
            # paced warm-up matmuls on landed data (results unused): keep the
            # PE HAM activity monitor busy through the DMA phase.
            p_w = ps_pool.tile([P, 512], FP32, tag="u", name=f"wu{ib}")
            for w in range(n_warm):
                nc.tensor.matmul(
                    p_w[:, (w % 4) * P:(w % 4 + 1) * P], ident[:],
                    xb[0:P, w * P:(w + 1) * P].bitcast(FP32), start=True, stop=True,
                )
            # transposes run in fp32r (1.5 cyc/row vs 2.0 for fp32)
            p_t = ps_pool.tile([P, N], FP32R, tag="big", name=f"ptp{ib}", bufs=3)
            for c in range(DC):
                nc.tensor.transpose(
                    p_t[:, c * P:(c + 1) * P],
                    xb[0:P, c * P:(c + 1) * P],
                    identr[:],
                )
            for c in range(DC):
                if c % 2 == 0:
                    nc.scalar.copy(xt[c][:, ib * P:(ib + 1) * P], p_t[:, c * P:(c + 1) * P])
                else:
                    nc.vector.tensor_copy(xt[c][:, ib * P:(ib + 1) * P], p_t[:, c * P:(c + 1) * P])

        def emit_qk_proj_half(key, dst, mb, ih):
            w4 = w4q[(key, mb)]
            p_t = ps_pool.tile([P, N], FP32, tag="big", name=f"pp{key}{mb}_{ih}", bufs=3)
            for c in range(DC):
                nc.tensor.matmul(
                    p_t[:, 0:512],
                    w4[:, c, :],
                    xt[c][:, ih * 512:(ih + 1) * 512],
                    start=(c == 0), stop=(c == DC - 1),
                )
            nc.vector.tensor_copy(dst[mb][:, ih * 512:(ih + 1) * 512], p_t[:, 0:512])

        def emit_v_proj(jb):
            p_t = ps_pool.tile([P, N], FP32, tag="big", name=f"pv{jb}", bufs=3)
            for c in range(DC):
                nc.tensor.matmul(
                    p_t[:, 0:512],
                    xt[c][:, jb * P:(jb + 1) * P],
                    wv_t[:, c, :],
                    start=(c == 0), stop=(c == DC - 1),
                )
            nc.vector.tensor_copy(vts[jb][:], p_t[:, 0:512])

        # DMA priority: q0/k0 quarters + bo ride the scalar queue (small,
        # needed first); x streams as 8 full-block [128,1024] tiles split
        # across the sync AND vector queues (dma_start issue costs ~0.8us
        # per instruction per queue, and two queues double the stream rate);
        # wv and the q1/k1 quarters ride the gpsimd queue.
        w4q[("q", 0)] = load_qk_quarter("q", wq_d, 0)
        w4q[("k", 0)] = load_qk_quarter("k", wk_d, 0)
        xblocks = {}

        def emit_x_dma(ib):
            xb = xn_pool.tile([P, N], FP32R, tag="xn", name=f"xn{ib}", bufs=3)
            eng = nc.sync if ib % 2 == 0 else nc.gpsimd
            eng.dma_start(
                out=xb[:],
                in_=x_d.ap()[ib * P:(ib + 1) * P, :].bitcast(FP32R),
            )
            xblocks[ib] = xb

        for ib in range(4):
            emit_x_dma(ib)
        wv_t = wv_pool.tile([P, DC, INNER], FP32R, tag="wv")
        nc.scalar.dma_start(
            out=wv_t[:],
            in_=wv_d.ap().rearrange("(c p) n -> p c n", p=P).bitcast(FP32R),
        )
        for ib in range(NB):
            # stagger the tail x DMAs so a queue's dma_start is never ordered
            # ahead of the drain copies its xn slot transitively waits on
            if ib < 4:
                emit_x_dma(ib + 4)
            emit_x_block(ib, n_warm=2, xb=xblocks[ib])
            if ib == 1:
                # one contiguous ~4-5us burst of real matmuls on the landed
                # blocks: trips the HAM SHORT window so the rest of the
                # preamble (transposes/projections) runs at 2.4 GHz.
                p_wb = ps_pool.tile([P, 512], FP32, tag="u", name="wburst")
                for wb in range(4):
                    nc.tensor.matmul(
                        p_wb[:], ident[:],
                        xblocks[wb % 2][0:P, (wb // 2) * 512:(wb // 2 + 1) * 512].bitcast(FP32),
                        start=True, stop=True,
                    )
            if ib == 3:
                # the i-half-0 projections only need blocks 0-3: run them in
                # the DMA-wait window of blocks 4-7
                emit_qk_proj_half("q", qt, 0, 0)
                emit_qk_proj_half("k", kt, 0, 0)
        w4q[("q", 1)] = load_qk_quarter("q", wq_d, 1)
        w4q[("k", 1)] = load_qk_quarter("k", wk_d, 1)
        emit_qk_proj_half("q", qt, 0, 1)
        emit_qk_proj_half("k", kt, 0, 1)

        # ---------------- head-pair loop ----------------
        finish_args = {}

        def emit_finish_p1(hA, hB):
            # 1/R via a PE-transpose reshape: R [1,1024] -> [128,8] so the
            # DVE iterative divide runs on 128 lanes.  Both heads share one
            # psum tile and one reciprocal: pu1[:,0:8]=A, [:,8:16]=B.
            usA, usB = finish_args[hA], finish_args[hB]
            pu1 = ps_pool.tile([P, 512], FP32, tag="u", name=f"f1_{hA}")
            for ib in range(NB):
                nc.tensor.matmul(
                    pu1[:, ib:ib + 1], usA[DH:DH + 1, ib * P:(ib + 1) * P],
                    ident[DH:DH + 1, DH:DH + 1], start=True, stop=True,
                )
            for ib in range(NB):
                nc.tensor.matmul(
                    pu1[:, 8 + ib:9 + ib], usB[DH:DH + 1, ib * P:(ib + 1) * P],
                    ident[DH:DH + 1, DH:DH + 1], start=True, stop=True,
                )
            rs = sm_pool.tile([P, 16], FP32, tag="rs", name=f"rs{hA}")
            nc.vector.reciprocal(rs[:], pu1[:, 0:16])
            finish_args[("rs", hA)] = rs

        def emit_finish_p2(hA, hB):
            rs = finish_args.pop(("rs", hA))
            for h in (hA, hB):
                us = finish_args.pop(h)
                mb, off = h // 2, (h % 2) * DH
                co = (h % 2) * 8
                rr = sm_pool.tile([1, N], FP32, tag="rr", name=f"rr{h}", bufs=1)
                for g in range(2):
                    pu2 = ps_pool.tile([P, 512], FP32, tag="u", name=f"f2_{h}_{g}")
                    for q in range(4):
                        nc.tensor.transpose(
                            pu2[0:1, q * P:(q + 1) * P],
                            rs[:, co + g * 4 + q:co + g * 4 + q + 1],
                            ident[:],
                        )
                    nc.vector.tensor_copy(rr[:, g * 512:(g + 1) * 512], pu2[0:1, 0:512])
                bc = sm_pool.tile([DH, N], FP32, tag="bc", name=f"bc{h}", bufs=1)
                nc.gpsimd.partition_broadcast(bc[:], rr[:])
                nc.vector.tensor_mul(ot[mb][off:off + DH, :], us[0:DH, :], bc[:])

        def emit_finish_pair(hA, hB):
            emit_finish_p1(hA, hB)
            emit_finish_p2(hA, hB)

        pair_state = {}

        def emit_scores(p, jb):
            mb = p
            psA = ps_pool.tile([P, N], FP32, tag="big", name=f"sA{p}_{jb}", bufs=3)
            psB = ps_pool.tile([P, N], FP32, tag="big", name=f"sB{p}_{jb}", bufs=3)
            # both i-halves of a head together: consecutive matmuls share the
            # same stationary operand (one weight load serves both)
            for ih in range(2):
                nc.tensor.matmul(
                    psA[:, ih * 512:(ih + 1) * 512],
                    kt[mb][0:DH, jb * P:(jb + 1) * P],
                    qt[mb][0:DH, ih * 512:(ih + 1) * 512],
                    start=True, stop=True,
                )
            for ih in range(2):
                nc.tensor.matmul(
                    psB[:, ih * 512:(ih + 1) * 512],
                    kt[mb][DH:P, jb * P:(jb + 1) * P],
                    qt[mb][DH:P, ih * 512:(ih + 1) * 512],
                    start=True, stop=True,
                )
            pair_state[(p, jb)] = (psA, psB)

        def emit_link(p, par, jb):
            """One jb link of head (2p+par)'s i-half-0 chain."""
            st = pair_state[("tiles", p)]
            v2t = st["v2A"] if par == 0 else st["v2B"]
            ett = st["etA"] if par == 0 else st["etB"]
            pu = st["puA"] if par == 0 else st["puB"]
            nc.tensor.matmul(
                pu[0:DH + 1, :],
                v2t[:, jb, 0:DH + 1],
                ett[jb][:, 0:512],
                start=(jb == 0), stop=(jb == NB - 1),
            )

        def emit_ih1_burst(p, par):
            """i-half-1 chain for head (2p+par): 8 matmuls into the slot the
            half-0 drain just freed, then drain into us[:, 512:1024]."""
            st = pair_state[("tiles", p)]
            v2t = st["v2A"] if par == 0 else st["v2B"]
            ett = st["etA"] if par == 0 else st["etB"]
            h = 2 * p + par
            us = finish_args[h]
            pu = ps_pool.tile([P, 512], FP32, tag="u", name=f"pu1_{h}")
            for jb in range(NB):
                nc.tensor.matmul(
                    pu[0:DH + 1, :],
                    v2t[:, jb, 0:DH + 1],
                    ett[jb][:, 512:1024],
                    start=(jb == 0), stop=(jb == NB - 1),
                )
            nc.vector.tensor_copy(us[:, 512:1024], pu[0:DH + 1, :])

        def emit_exps(p, jb):
            """exp + C + v2 normalize for both heads of pair p, block jb."""
            st = pair_state[("tiles", p)]
            hA, hB = 2 * p, 2 * p + 1
            cA, cB = st["cA"], st["cB"]
            v2A, v2B = st["v2A"], st["v2B"]
            etA, etB = st["etA"], st["etB"]
            psA, psB = pair_state.pop((p, jb))
            # jb-0 tiles are double-buffered: the next pair's first exps
            # race the previous pair's i-half-1 bursts (which read these).
            eb = 2 if jb < 1 else 1
            etA[jb] = et_pool.tile([P, N], BF16, tag=f"etA{jb}", name=f"etA{p}_{jb}", bufs=eb)
            nc.scalar.activation(
                etA[jb][:], psA[:], mybir.ActivationFunctionType.Exp,
                scale=SCALE, accum_out=cA[:, jb:jb + 1],
            )
            nc.gpsimd.normalize_recip(
                v2A[:, jb, 0:DH], vts[jb][:, hA * DH:(hA + 1) * DH], cA[:, jb:jb + 1]
            )
            nc.gpsimd.tensor_copy(v2A[:, jb, DH:DH + 1], cA[:, jb:jb + 1])
            etB[jb] = et_pool.tile([P, N], BF16, tag=f"etB{jb}", name=f"etB{p}_{jb}", bufs=eb)
            nc.scalar.activation(
                etB[jb][:], psB[:], mybir.ActivationFunctionType.Exp,
                scale=SCALE, accum_out=cB[:, jb:jb + 1],
            )
            nc.gpsimd.normalize_recip(
                v2B[:, jb, 0:DH], vts[jb][:, hB * DH:(hB + 1) * DH], cB[:, jb:jb + 1]
            )
            nc.gpsimd.tensor_copy(v2B[:, jb, DH:DH + 1], cB[:, jb:jb + 1])

        def open_pair(p):
            hA, hB = 2 * p, 2 * p + 1
            cA = c_pool.tile([P, NB], FP32, tag="cA", name=f"cA{p}")
            cB = c_pool.tile([P, NB], FP32, tag="cB", name=f"cB{p}")
            v2A = v2_pool.tile([P, NB, DH + 2], BF16, tag="v2A", name=f"v2A{p}")
            v2B = v2_pool.tile([P, NB, DH + 2], BF16, tag="v2B", name=f"v2B{p}")
            puA = ps_pool.tile([P, 512], FP32, tag="u", name=f"pu0A_{p}")
            puB = ps_pool.tile([P, 512], FP32, tag="u", name=f"pu0B_{p}")
            pair_state[("tiles", p)] = {
                "v2A": v2A, "v2B": v2B, "etA": {}, "etB": {},
                "cA": cA, "cB": cB, "puA": puA, "puB": puB,
            }

        def close_half(p, par):
            """Link(7), half-0 drain, half-1 burst for one head of pair p.
            The A half runs under the pair's last exp (it only needs exp A7);
            the B half follows right after exp B7."""
            st = pair_state[("tiles", p)]
            h = 2 * p + par
            emit_link(p, par, NB - 1)
            us = us_pool.tile([DH + 1, N], FP32, tag="us", name=f"us{h}")
            finish_args[h] = us
            pu = st["puA"] if par == 0 else st["puB"]
            nc.vector.tensor_copy(us[:, 0:512], pu[0:DH + 1, :])
            emit_ih1_burst(p, par)

        # pair 0: seed two score emissions out of the preamble (the ring is
        # 3 big tiles = 1.5 emissions of lookahead); v0/v1 fill the rest.
        emit_scores(0, 0)
        emit_scores(0, 1)
        emit_v_proj(0)
        emit_v_proj(1)
        open_pair(0)

        # filler schedule: (pair, jb) -> list of callables.  Quarter p+1's
        # projections run early in pair p (they gate the next pair's scores,
        # which are now emitted in pair p's LATE jbs to keep the PE fed).
        fillers = {
            (0, 0): [lambda: emit_v_proj(2)],
            (0, 1): [lambda: emit_v_proj(3), lambda: emit_qk_proj_half("q", qt, 1, 0)],
            (0, 2): [lambda: emit_v_proj(4), lambda: emit_qk_proj_half("k", kt, 1, 0)],
            (0, 3): [lambda: emit_v_proj(5), lambda: emit_qk_proj_half("q", qt, 1, 1)],
            (0, 4): [lambda: emit_v_proj(6)],
            (0, 5): [lambda: emit_v_proj(7), lambda: emit_qk_proj_half("k", kt, 1, 1)],
            (1, 0): [lambda: emit_qk_proj_half("q", qt, 2, 0)],
            (1, 1): [lambda: emit_qk_proj_half("k", kt, 2, 0)],
            (1, 2): [lambda: emit_qk_proj_half("q", qt, 2, 1)],
            (1, 3): [lambda: emit_qk_proj_half("k", kt, 2, 1)],
            (2, 0): [lambda: emit_qk_proj_half("q", qt, 3, 0)],
            (2, 1): [lambda: emit_qk_proj_half("k", kt, 3, 0)],
            (2, 2): [lambda: emit_qk_proj_half("q", qt, 3, 1)],
            (2, 3): [lambda: emit_qk_proj_half("k", kt, 3, 1)],
        }

        # out-projection state: psum partials (mb0-2 + bo) drained into SBUF
        # tiles aliased onto the dead qt/kt slots; mb3 sweeps at the tail.
        yp = {}

        def emit_py_pre(ib):
            """Wo chunks mb0-2 + bo for block ib, drained to an SBUF partial
            aliased onto the (dead) qt/kt slots."""
            p_y = ps_pool.tile([P, N], FP32, tag="big", name=f"py{ib}", bufs=3)
            for db in range(2):
                for mbi in range(3):
                    nc.tensor.matmul(
                        p_y[:, db * 512:(db + 1) * 512],
                        ot[mbi][:, ib * P:(ib + 1) * P],
                        wo4[mbi][:, db * 512:(db + 1) * 512],
                        start=(mbi == 0), stop=(mbi == 2),
                    )
            pool = qt_pool if ib < 4 else kt_pool
            ypt = pool.tile([P, N], FP32R, tag=f"{'qt' if ib < 4 else 'kt'}{ib % 4}",
                            name=f"yp{ib}")
            yp[ib] = ypt
            for db in range(2):
                nc.vector.tensor_add(
                    ypt[:, db * 512:(db + 1) * 512],
                    p_y[:, db * 512:(db + 1) * 512],
                    bo_bc[:, db * 512:(db + 1) * 512],
                )

        def emit_py_mb3(ib):
            """mb3 matmuls + identity-matmul re-add of the SBUF partial (PE,
            fp32r), then plain drain copies split across ACT and DVE."""
            p_y = ps_pool.tile([P, N], FP32, tag="big", name=f"py3_{ib}", bufs=3)
            for db in range(2):
                nc.tensor.matmul(
                    p_y[:, db * 512:(db + 1) * 512],
                    ot[3][:, ib * P:(ib + 1) * P],
                    wo4[3][:, db * 512:(db + 1) * 512],
                    start=True, stop=False,
                )
                nc.tensor.matmul(
                    p_y[:, db * 512:(db + 1) * 512],
                    identr[:],
                    yp[ib][:, db * 512:(db + 1) * 512],
                    start=False, stop=True,
                )
            # ACT and DVE copy one half each in parallel; the two DMAs ride
            # different queues so the drains never serialize on one engine.
            for db in range(2):
                y_t = y_pool.tile([P, 512], FP32, tag="y", name=f"y{ib}_{db}")
                if db == 0:
                    nc.scalar.copy(y_t[:], p_y[:, 0:512])
                else:
                    nc.vector.tensor_copy(y_t[:], p_y[:, 512:1024])
                (nc.sync if db == 0 else nc.gpsimd).dma_start(
                    out=y_d.ap()[ib * P:(ib + 1) * P, db * 512:(db + 1) * 512],
                    in_=y_t[:],
                )

        for p in range(4):
            for jb in range(NB):
                emit_exps(p, jb)
                # PE: links one jb behind the exps, then fillers, then the
                # ring-ahead scores so a slot wait never heads the queue.
                if jb > 0:
                    emit_link(p, 0, jb - 1)
                    emit_link(p, 1, jb - 1)
                for f in fillers.get((p, jb), ()):
                    f()
                if jb <= 5:
                    emit_scores(p, jb + 2)
                elif jb == 6 and p < 3:
                    # late jbs build the NEXT pair's first scores so the PE
                    # never idles into a HAM re-throttle at the pair tail.
                    emit_scores(p + 1, 0)
                if p == 3 and jb >= 5:
                    emit_py_pre(jb - 5)  # blocks 0-2 while the exps drain

            if p == 2:
                # xt is dead (last qk proj half was this pair); Wo quarters
                # reuse the xt slots and land well before the tail.
                wo4 = []
                for mbi in range(IC):
                    w4 = xt_pool.tile([P, N], FP32R, tag=f"xt{mbi}", name=f"wo{mbi}")
                    nc.sync.dma_start(
                        out=w4[:], in_=wo_d.ap()[mbi * P:(mbi + 1) * P, :].bitcast(FP32R)
                    )
                    wo4.append(w4)

            if p < 3:
                # boundary: the A half-1 burst runs under exp(B,7); the next
                # pair's second scores slot between the closes so the exp
                # stream rolls through the boundary without a gap.
                close_half(p, 0)
                emit_scores(p + 1, 1)
                close_half(p, 1)
                emit_finish_pair(2 * p, 2 * p + 1)
                open_pair(p + 1)
            else:
                # ---------------- tail ----------------
                # finishes first (their DVE/gpsimd chain runs while the PE
                # streams the py pre-accumulations), then the mb3 sweep.
                close_half(3, 0)
                close_half(3, 1)
                emit_finish_pair(H - 2, H - 1)
                for ib in range(3, NB):
                    emit_py_pre(ib)
                for ib in range(NB):
                    emit_py_mb3(ib)

            if p == 0:
                w4q[("q", 2)] = load_qk_quarter("q", wq_d, 2, eng=nc.sync)
                w4q[("k", 2)] = load_qk_quarter("k", wk_d, 2, eng=nc.sync)
            if p == 1:
                w4q[("q", 3)] = load_qk_quarter("q", wq_d, 3, eng=nc.sync)
                w4q[("k", 3)] = load_qk_quarter("k", wk_d, 3, eng=nc.sync)

        for pool in (ps_pool, y_pool, sm_pool, us_pool, c_pool, v2_pool, et_pool,
                     xn_pool, w4_pool, wv_pool, ot_pool, v_pool, kt_pool, qt_pool,
                     xt_pool, const_pool):
            pool.release()

    nc.finalize()
    return nc


def _get_nc():
    global _NC_CACHE
    if _NC_CACHE is None:
        _NC_CACHE = _build_nc()
    return _NC_CACHE


def kernel(x, Wq, Wk, Wv, Wo, bo, _trace=False, **trace_kwargs):
    x = np.ascontiguousarray(np.asarray(x, dtype=np.float32))
    Wq = np.ascontiguousarray(np.asarray(Wq, dtype=np.float32))
    Wk = np.ascontiguousarray(np.asarray(Wk, dtype=np.float32))
    Wv = np.ascontiguousarray(np.asarray(Wv, dtype=np.float32))
    Wo = np.ascontiguousarray(np.asarray(Wo, dtype=np.float32))
    bo = np.ascontiguousarray(np.asarray(bo, dtype=np.float32))

    nc = _get_nc()
    in_maps = [
        {"x": x[c], "Wq": Wq, "Wk": Wk, "Wv": Wv, "Wo": Wo, "bo": bo}
        for c in range(NCORES)
    ]
    res = run_bass_kernel_spmd(
        nc, in_maps, core_ids=list(range(NCORES)), trace=_trace, **trace_kwargs
    )
    out = np.stack([res.results[c]["y"] for c in range(NCORES)], axis=0)
    if _trace:
        return out.astype(np.float32), res
    return out.astype(np.float32)


if __name__ == "__main__":
    rng = np.random.default_rng(0)
    xs = rng.standard_normal((B, N, D), dtype=np.float32)
    wq = rng.standard_normal((D, INNER), dtype=np.float32) * D ** -0.5
    wk = rng.standard_normal((D, INNER), dtype=np.float32) * D ** -0.5
    wv = rng.standard_normal((D, INNER), dtype=np.float32) * D ** -0.5
    wo = rng.standard_normal((INNER, D), dtype=np.float32) * INNER ** -0.5
    bz = np.zeros((D,), dtype=np.float32)
    y = kernel(xs, wq, wk, wv, wo, bz)
    print("ran ok", y.shape, float(np.abs(y).mean()))
